# revision 1
# baseline (speedup 1.0000x reference)
"""CAM (channel attention module) kernel for Trainium2, 8-core SPMD.

Problem: x (16, 512, 64, 64) f32, gamma (1,) f32.
  v = x.reshape(B, C, N);  E = v @ v.T  (B x 512 x 512)
  att = softmax(rowmax(E) - E)  ==  exp(rowmin(E) - E) / rowsum(...)
  out = gamma * (att @ v) + x

Sharding: data-parallel over batch, 2 batches per core, no collectives.

Per-core per-batch pipeline (matmul operands in fp16 = TF32-class accuracy,
all accumulation and the x-residual in exact f32). Tile-dependency tracking
is whole-tile, so tensors are split into tiles matching their consumers'
granularity (v in quarters, att per row-tile, attT per d-tile):
  T: DMA v quarters (f32, spread over sync/gpsimd/scalar DMA queues,
     v0/v1/v2 double-buffered cross-batch) -> gpsimd/DVE f32->fp16 copies
     per quarter -> 128 PE transposes (fp16, 1 cy/row) -> DVE copies to vT
     (one full fp16 PSUM bank = 8 transposes = one quarter).
  E: energy = vT.T @ vT, fp16 operands, f32 PSUM accumulation. Symmetry:
     row-tile ct computes only columns >= ct*128 (N=512/384/256/128, the
     upper triangle); the 6 missing blocks are mirrored from earlier rows
     by PE transpose straight back into the PSUM row.
  S: fused softmax of (rowmin(E) - E) [stable form of softmax(rowmax-E)]:
     DVE rowmin, ACT exp with accumulated rowsum, DVE reciprocal; gamma is
     folded into the per-row scale, so gamma==0 gives att==0 and the output
     is bit-exact x.
  A: 16 PE transposes att -> attT (fp16), per-dt tiles.
  O: out = attT.T @ chunks(fp16 of v) accumulated over d-chunks into f32
     PSUM; DVE epilogue adds the exact-f32 x tile; 256KB stores. The last
     n-iter reads x for ct3 from a re-streamed tile so v3's last quarter
     releases early for the next batch's load.
"""
import sys

import numpy as np

if "/opt/trn_rl_repo" not in sys.path:
    sys.path.insert(0, "/opt/trn_rl_repo")

import concourse.bass as bass
import concourse.tile as tile
from concourse import bacc, mybir
from concourse.bass_utils import run_bass_kernel_spmd
from concourse.masks import make_identity

N_CORES = 8
B_FULL = 16
B_PER_CORE = B_FULL // N_CORES  # 2
C = 512            # channels
HW = 4096          # H*W
CT = C // 128      # 4 channel tiles
KCH = HW // 128    # 32 contraction chunks for energy
NCH = HW // 512    # 8 output column chunks
QW = HW // 4       # quarter of H*W (v quarter-tile width)

f32 = mybir.dt.float32
f16 = mybir.dt.float16

_CACHE = {}

# engine that converts f32->fp16 for each ct's halves in the T phase
CONV_ENGINES = {0: "vector", 1: "gpsimd", 2: "gpsimd", 3: "gpsimd"}
# DMA queue that loads each ct's halves
LOAD_ENGINES = {0: "sync", 1: "gpsimd", 2: "scalar", 3: "scalar"}


def _build_nc(reps: int = 1):
    nc = bacc.Bacc(None, target_bir_lowering=False)
    x_d = nc.dram_tensor("x", [B_PER_CORE, C, HW], f32, kind="ExternalInput")
    g_d = nc.dram_tensor("gamma", [1], f32, kind="ExternalInput")
    y_d = nc.dram_tensor("y", [B_PER_CORE, C, HW], f32, kind="ExternalOutput")

    with tile.TileContext(nc) as tc:
        with (
            tc.tile_pool(name="pvA", bufs=2) as pvA,        # v0/v1/v2 halves
            tc.tile_pool(name="pvB", bufs=1) as pvB,        # v3 halves
            tc.tile_pool(name="pvt", bufs=1) as pvt,        # vT fp16 32KB
            tc.tile_pool(name="pv16", bufs=2) as pv16,      # fp16 v half-tiles
            tc.tile_pool(name="patt", bufs=1) as patt,      # att/attT fp16
            tc.tile_pool(name="pchunk", bufs=2) as pchunk,  # fp16 rhs chunks
            tc.tile_pool(name="pstage", bufs=4) as pstage,  # out staging
            tc.tile_pool(name="pstream", bufs=1) as pstream,  # x re-stream
            tc.tile_pool(name="psmall", bufs=8) as psmall,  # per-ct scalars
            tc.tile_pool(name="pmir", bufs=1) as pmir,      # mirror blocks
            tc.tile_pool(name="psing", bufs=1) as psing,    # ident, gamma
            tc.tile_pool(name="ptp", bufs=2, space="PSUM") as ptp,
            tc.tile_pool(name="pep", bufs=2, space="PSUM") as pep,
            tc.tile_pool(name="pop", bufs=4, space="PSUM") as pop,
        ):
            ident = psing.tile([128, 128], f32)
            make_identity(nc, ident)
            ident16 = psing.tile([128, 128], f16)
            nc.vector.tensor_copy(out=ident16, in_=ident)
            gam = psing.tile([128, 1], f32)

            def load_gamma():
                g_ap = g_d[:]
                nc.gpsimd.dma_start(
                    out=gam,
                    in_=bass.AP(tensor=g_ap.tensor, offset=g_ap.offset,
                                ap=[[0, 128], [1, 1]]),
                )

            def load_vq(b, ct, q):
                pool = pvB if ct == 3 else pvA
                t_ = pool.tile([128, QW], f32, tag=f"v{ct}q{q}",
                               name=f"v{ct}q{q}")
                getattr(nc, LOAD_ENGINES[ct]).dma_start(
                    out=t_,
                    in_=x_d[b, ct * 128:(ct + 1) * 128, q * QW:(q + 1) * QW],
                )
                return t_

            def load_group(b):
                tiles = [[None] * 4 for _ in range(3)]
                for q in range(4):
                    for ct in (0, 1, 2):
                        tiles[ct][q] = load_vq(b, ct, q)
                return tiles

            batches = [bb for _ in range(reps) for bb in range(B_PER_CORE)]
            v012 = load_group(batches[0])


            for bi, b in enumerate(batches):
                v = list(v012) + [[load_vq(b, 3, q) for q in range(4)]]

                def xcol(dt, n):
                    """f32 x slice [128, 512] for (row-tile dt, n-chunk n)."""
                    q, lo = divmod(n * 512, QW)
                    return v[dt][q][:, lo:lo + 512]

                # ---- T: fp16 convert + PE transposes (1 cy/row).
                # vT is split into 4 k-quarter tiles so the energy chains
                # can start on early quarters while the last quarters'
                # PSUM->SBUF copies are still draining on DVE.
                vTq = [pvt.tile([128, 8, C], f16, tag=f"vTq{q}",
                                name=f"vTq{q}") for q in range(4)]

                def vT(k):
                    return vTq[k // 8][:, k % 8, :]

                for ct in range(CT):
                    eng = getattr(nc, CONV_ENGINES[ct])
                    for q in range(4):
                        v16 = pv16.tile([128, QW], f16, tag=f"v16_{q % 2}")
                        eng.tensor_copy(out=v16, in_=v[ct][q])
                        tp = ptp.tile([128, 8, 128], f16, tag="tp")
                        for ks in range(8):
                            nc.tensor.transpose(
                                tp[:, ks, :],
                                v16[:, ks * 128:(ks + 1) * 128],
                                ident16,
                            )
                        nc.vector.tensor_copy(
                            out=vTq[q][:, :, ct * 128:(ct + 1) * 128],
                            in_=tp,
                        )
                if bi == 0:
                    load_gamma()  # emitted late so it never delays v loads
                # prefetch next batch's double-buffered v0/v1/v2
                if bi + 1 < len(batches):
                    v012 = load_group(batches[bi + 1])

                # ---- E + S: energy (fp16 operands, f32 accum) + softmax ----
                att = [None] * CT
                mirror_src = {}  # (ct, dt) -> sbuf [128,128] copy of E block
                for ct in range(CT):
                    off = ct * 128
                    ep = pep.tile([128, C], f32, tag="ep")
                    for k in range(KCH):
                        nc.tensor.matmul(
                            ep[:, off:],
                            lhsT=vT(k)[:, ct * 128:(ct + 1) * 128],
                            rhs=vT(k)[:, off:],
                            start=(k == 0),
                            stop=(k == KCH - 1),
                        )
                    # stash blocks that later rows mirror
                    for (dst, src) in (((1, 0), (0, 1)), ((2, 0), (0, 2)),
                                       ((2, 1), (1, 2)), ((3, 0), (0, 3)),
                                       ((3, 1), (1, 3)), ((3, 2), (2, 3))):
                        if src[0] == ct:
                            sb = pmir.tile([128, 128], f32,
                                           tag=f"mir{dst[0]}{dst[1]}")
                            nc.vector.tensor_copy(
                                out=sb,
                                in_=ep[:, src[1] * 128:(src[1] + 1) * 128],
                            )
                            mirror_src[dst] = sb
                    for dt in range(ct):
                        nc.tensor.transpose(
                            ep[:, dt * 128:(dt + 1) * 128],
                            mirror_src[(ct, dt)], ident,
                        )
                    mn = psmall.tile([128, 1], f32, tag="mn")
                    nc.vector.tensor_reduce(
                        out=mn, in_=ep, axis=mybir.AxisListType.X,
                        op=mybir.AluOpType.min,
                    )
                    a_ = patt.tile([128, C], f16, tag=f"att{ct}")
                    ss = psmall.tile([128, 1], f32, tag="ss")
                    nc.scalar.activation(
                        out=a_, in_=ep,
                        func=mybir.ActivationFunctionType.Exp,
                        bias=mn, scale=-1.0, accum_out=ss,
                    )
                    rg = psmall.tile([128, 1], f32, tag="rg")
                    nc.vector.reciprocal(out=rg, in_=ss)
                    nc.vector.tensor_mul(out=rg, in0=rg, in1=gam)
                    nc.vector.tensor_scalar_mul(a_, a_, rg)
                    att[ct] = a_

                # ---- A: transpose att -> attT (fp16, per-dt tiles).
                # The 4 transpose banks live in the (idle) O-phase PSUM
                # slots; all ct0..2 transposes are emitted first so they
                # execute while softmax(ct3) is still finishing on DVE/ACT.
                atp = [pop.tile([128, CT, 128], f16, tag="op",
                                name=f"atp{dt}") for dt in range(CT)]
                for ct in range(CT - 1):
                    for dt in range(CT):
                        nc.tensor.transpose(
                            atp[dt][:, ct, :],
                            att[ct][:, dt * 128:(dt + 1) * 128],
                            ident16,
                        )
                attT = []
                for dt in range(CT):
                    nc.tensor.transpose(
                        atp[dt][:, CT - 1, :],
                        att[CT - 1][:, dt * 128:(dt + 1) * 128],
                        ident16,
                    )
                    aT = patt.tile([128, CT, 128], f16, tag=f"attT{dt}")
                    nc.vector.tensor_copy(out=aT, in_=atp[dt])
                    attT.append(aT)

                # ---- O: out = attT.T @ chunks + x, per 512-wide n-chunk ----
                # last n-iter x for ct3 comes from a re-streamed tile so
                # v3's second half releases one iteration early
                xs3 = pstream.tile([128, 512], f32, tag="xs3")
                nc.sync.dma_start(
                    out=xs3, in_=x_d[b, 384:512, (NCH - 1) * 512:],
                )

                def xsrc(dt, n):
                    if n == NCH - 1 and dt == 3:
                        return xs3
                    return xcol(dt, n)

                for n in range(NCH):
                    nsl = slice(n * 512, (n + 1) * 512)
                    chunks = []
                    for dt in range(CT):
                        ch = pchunk.tile([128, 512], f16, tag=f"ch{dt}")
                        nc.gpsimd.tensor_copy(out=ch, in_=xsrc(dt, n))
                        chunks.append(ch)
                    for ct in range(CT):
                        op = pop.tile([128, 512], f32, tag="op")
                        for dt in range(CT):
                            nc.tensor.matmul(
                                op,
                                lhsT=attT[dt][:, ct, :],
                                rhs=chunks[dt],
                                start=(dt == 0),
                                stop=(dt == CT - 1),
                            )
                        st = pstage.tile([128, 512], f32, tag="st")
                        nc.vector.tensor_add(out=st, in0=op, in1=xsrc(ct, n))
                        nc.sync.dma_start(
                            out=y_d[b, ct * 128:(ct + 1) * 128, nsl], in_=st,
                        )

    nc.compile()
    return nc


def kernel(x: np.ndarray, gamma: np.ndarray) -> np.ndarray:
    x = np.ascontiguousarray(np.asarray(x, dtype=np.float32))
    gamma = np.ascontiguousarray(np.asarray(gamma, dtype=np.float32))
    B, Cc, H, W = x.shape
    xv = x.reshape(B, Cc, H * W)

    if "nc" not in _CACHE:
        _CACHE["nc"] = _build_nc()
    nc = _CACHE["nc"]

    in_maps = [
        {"x": xv[i * B_PER_CORE:(i + 1) * B_PER_CORE], "gamma": gamma}
        for i in range(N_CORES)
    ]
    res = run_bass_kernel_spmd(nc, in_maps, list(range(N_CORES)))
    y = np.concatenate([res.results[i]["y"] for i in range(N_CORES)], axis=0)
    return y.reshape(B, Cc, H, W).astype(np.float32)



# revision 54
# speedup vs baseline: 1.1015x; 1.1015x over previous
"""CAM (channel attention module) kernel for Trainium2, 8-core SPMD.

Problem: x (16, 512, 64, 64) f32, gamma (1,) f32.
  v = x.reshape(B, C, N);  E = v @ v.T  (B x 512 x 512)
  att = softmax(rowmax(E) - E)  ==  exp(rowmin(E) - E) / rowsum(...)
  out = gamma * (att @ v) + x
Sharding: data-parallel over batch, 2 batches per core, no collectives.

Cost-model facts this schedule is built around (CoreSim legacy model):
  - Each DMA queue is per-engine and serial WITH that engine's compute
    track; cost = bytes-per-partition * 0.386ns. SP has no compute: it
    carries half the stores and ALL of the next batch's prefetched loads
    (so prefetch never blocks the ACT/Pool compute tracks).
  - PE matmul cost = out_free * cycles/row (fp16 1.0, fp8e4 DoubleRow 0.5
    with a 256-deep contraction, fp32r 1.0 at free>=256, transpose fp16
    1.0). PE total is ~73us for 2 batches - the critical path.
  - Engine elementwise cost ~ elements * cycle (DVE 1.04, ACT 0.83, Pool
    1.39 effective). fp16->fp16 copies get 2x on DVE.

Per-core per-batch pipeline (quarter-granular so the conversion feed
always runs ahead of the in-order PE):
  T: DMA v quarters (f32 [128,1024], q-major, round-robin queues;
     next batch prefetched on sync during this batch's T phase) ->
     f32->fp16 quarter copies (DVE/ACT/Pool round-robin) -> 8 PE
     transposes -> one copy into that quarter's vT tile.
  E: energy = vT.T @ vT, fp16 operands, f32 PSUM accumulation, upper
     triangle only; the 6 lower blocks are mirrored from earlier rows by
     PE transpose back into the PSUM row. Rows 0/1 stream per quarter
     right behind the transposes; rows 2/3 + mirrors run after quarter 3,
     reusing rows 0/1's PSUM banks.
  S: fused softmax of (rowmin(E) - E): DVE rowmin, ACT exp with rowsum,
     DVE reciprocal; gamma is folded into the per-row scale, so gamma==0
     gives att==0 and the output is exactly x.
  A: per d-tile: 4 PE transposes att -> PSUM (fp16) + one copy that
     downcasts into fp8e4 pair tiles (two d-tiles packed for DoubleRow).
     Emitted after the previous batch's leftover O chunks so those fill
     the softmax wait on the in-order PE.
  O: per (ct, 1024-wide column pair): two 512-wide PSUM groups, each two
     fp8e4 DoubleRow matmuls over d=512. Chunks rotate epilogue engines:
     DVE chunks use fused tensor_add (op + x); ACT/Pool chunks accumulate
     the exact-f32 x residual via an fp32r identity matmul on PE and then
     do a plain PSUM->SBUF copy. Stores spread over sync/scalar/gpsimd.
  O(b) is interleaved with TE(b+1) quanta at emission (TE quantum first,
     so the next batch's chain-critical conversions get queue priority
     over the epilogue, which has slack).
"""
import sys

import numpy as np

if "/opt/trn_rl_repo" not in sys.path:
    sys.path.insert(0, "/opt/trn_rl_repo")

import concourse.bass as bass
import concourse.tile as tile
from concourse import bacc, mybir
from concourse.bass_utils import run_bass_kernel_spmd
from concourse.masks import make_identity

N_CORES = 8
B_FULL = 16
B_PER_CORE = B_FULL // N_CORES  # 2
C = 512            # channels
HW = 4096          # H*W
CT = C // 128      # 4 channel tiles
KCH = HW // 128    # 32 contraction chunks for energy
QW = HW // 4       # quarter of H*W
NN = 4             # 1024-wide output column pairs per row tile

f32 = mybir.dt.float32
f32r = mybir.dt.float32r
f16 = mybir.dt.float16
f8 = mybir.dt.float8e4
DR = mybir.MatmulPerfMode.DoubleRow

_CACHE = {}

# DMA queue per (ct, q) for this batch's loads; next batch: quarters 0/1
# prefetched on sync, quarters 2/3 spread late (after this batch's
# conversions clear the ACT/Pool queues)
LOAD_Q = ("sync", "scalar", "gpsimd", "sync")
# f32->fp16 quarter conversion engine per (q*4+ct)
V16_E = ("vector", "gpsimd", "scalar", "vector",
         "gpsimd", "scalar", "vector", "gpsimd",
         "scalar", "gpsimd", "vector", "scalar",
         "gpsimd", "scalar", "vector", "gpsimd")
# vT PSUM->SBUF copy engine per (q*4+ct)
VT_E = ("vector", "scalar", "vector", "scalar",
        "scalar", "vector", "scalar", "vector",
        "vector", "scalar", "vector", "scalar",
        "scalar", "vector", "scalar", "vector")
# f32->fp8 quarter conversion engine per (q*4+ct) (not chain-critical)
V8_E = ("gpsimd", "scalar", "gpsimd", "gpsimd",
        "scalar", "gpsimd", "gpsimd", "scalar",
        "gpsimd", "gpsimd", "scalar", "gpsimd",
        "gpsimd", "scalar", "gpsimd", "gpsimd")
# epilogue per (ct + nn) % 2: DVE does fused tensor_add(op + x); ACT
# chunks accumulate the exact-f32 x residual via an fp32r identity matmul
# on PE then do a plain PSUM->SBUF copy (GPSIMD cannot touch PSUM, and
# ACT cannot tensor-add, so att is pre-scaled by gamma/rowsum on DVE)
EPI_SEQ = (("vector", False), ("vector", False))
# store queue per (ct, nn)
STORE_Q = {3: ("sync", "sync", "gpsimd", "sync"),
           0: ("sync", "gpsimd", "sync", "sync"),
           1: ("sync", "sync", "scalar", "gpsimd"),
           2: ("sync", "sync", "gpsimd", "sync")}
MIR_E = "vector"  # mirror stashes read PSUM: GPSIMD cannot
AT8_E = ("scalar", "vector", "scalar", "vector")  # attT fp8 pack per dt


def _copy(nc, eng, out, in_):
    if eng == "scalar":
        nc.scalar.copy(out=out, in_=in_)
    else:
        getattr(nc, eng).tensor_copy(out=out, in_=in_)


def _build_nc(reps: int = 1):
    nc = bacc.Bacc(None, target_bir_lowering=False)
    x_d = nc.dram_tensor("x", [B_PER_CORE, C, HW], f32, kind="ExternalInput")
    g_d = nc.dram_tensor("gamma", [1], f32, kind="ExternalInput")
    y_d = nc.dram_tensor("y", [B_PER_CORE, C, HW], f32, kind="ExternalOutput")

    with tile.TileContext(nc) as tc:
        with (
            tc.tile_pool(name="pv", bufs=2) as pv,          # v quarters f32
            tc.tile_pool(name="pvt", bufs=1) as pvt,        # vT fp16
            tc.tile_pool(name="pv16", bufs=1) as pv16,      # fp16 v quarters
            tc.tile_pool(name="patt", bufs=1) as patt,      # att fp16
            tc.tile_pool(name="pat8", bufs=1) as pat8,      # attT fp8 pairs
            tc.tile_pool(name="pv8", bufs=1) as pv8,        # v fp8 pairs
            tc.tile_pool(name="pstage", bufs=4) as pstage,  # out staging
            tc.tile_pool(name="psmall", bufs=8) as psmall,  # per-ct scalars
            tc.tile_pool(name="pmir", bufs=1) as pmir,      # mirror blocks
            tc.tile_pool(name="psing", bufs=1) as psing,    # ident, gamma
            tc.tile_pool(name="ptp", bufs=2, space="PSUM") as ptp,
            tc.tile_pool(name="pep", bufs=2, space="PSUM") as pep,
            tc.tile_pool(name="pop", bufs=2, space="PSUM") as pop,
        ):
            ident = psing.tile([128, 128], f32)
            make_identity(nc, ident)
            ident16 = psing.tile([128, 128], f16)
            nc.vector.tensor_copy(out=ident16, in_=ident)
            gam = psing.tile([128, 1], f32)

            def load_gamma():
                g_ap = g_d[:]
                nc.gpsimd.dma_start(
                    out=gam,
                    in_=bass.AP(tensor=g_ap.tensor, offset=g_ap.offset,
                                ap=[[0, 128], [1, 1]]),
                )

            def load_quarters(b, tiles, qs, spread):
                for q in qs:
                    for ct in range(CT):
                        t_ = pv.tile([128, QW], f32, tag=f"v{ct}q{q}",
                                     name=f"v{ct}q{q}")
                        q_ = LOAD_Q[(q + ct) % len(LOAD_Q)] if spread \
                            else "sync"
                        getattr(nc, q_).dma_start(
                            out=t_,
                            in_=x_d[b, ct * 128:(ct + 1) * 128,
                                    q * QW:(q + 1) * QW],
                        )
                        tiles[ct][q] = t_
                return tiles

            def load_group(b, spread=True):
                return load_quarters(
                    b, [[None] * 4 for _ in range(CT)], range(4), spread)

            batches = [bb for _ in range(reps) for bb in range(B_PER_CORE)]
            state = {"v": load_group(batches[0])}

            def v8_emit(te, qq):
                """fp8 pair copies of v quarter qq for the O phase. The
                tiles are allocated HERE (first call), after the previous
                batch's O chunks are fully emitted, so the pool's
                write-after-read hazard tracking sees every reader of the
                previous tiles before the new writers are scheduled."""
                v = te["v"]
                if te["v8"] is None:
                    te["v8"] = [pv8.tile([128, 2, HW], f8, tag=f"v8_{p_}",
                                         name=f"v8_{p_}") for p_ in range(2)]
                v8 = te["v8"]
                for ct in range(CT):
                    _copy(nc, V8_E[qq * 4 + ct],
                          v8[ct // 2][:, ct % 2, qq * QW:(qq + 1) * QW],
                          v[ct][qq])

            def te_quanta(bi, b, out, inline_v8):
                """Emit T+E+S for batch b, yielding at quantum boundaries
                for interleaving with the previous O phase."""
                v = state["v"]
                out["v"] = v
                vTq = [pvt.tile([128, 8, C], f16, tag=f"vTq{q}",
                                name=f"vTq{q}") for q in range(4)]

                def vT(k):
                    return vTq[k // 8][:, k % 8, :]

                out["v8"] = None
                ep = {}
                att = [None] * CT
                out["att"] = att
                mirror_src = {}

                def e_row(ct, ks):
                    off = ct * 128
                    for k in ks:
                        nc.tensor.matmul(
                            ep[ct][:, off:],
                            lhsT=vT(k)[:, off:off + 128],
                            rhs=vT(k)[:, off:],
                            start=(k == 0),
                            stop=(k == KCH - 1),
                        )

                def stash_mirrors(ct):
                    for (dst, src) in (((1, 0), (0, 1)), ((2, 0), (0, 2)),
                                       ((2, 1), (1, 2)), ((3, 0), (0, 3)),
                                       ((3, 1), (1, 3)), ((3, 2), (2, 3))):
                        if src[0] == ct:
                            sb = pmir.tile([128, 128], f32,
                                           tag=f"mir{dst[0]}{dst[1]}")
                            _copy(nc, MIR_E, sb,
                                  ep[ct][:, src[1] * 128:(src[1] + 1) * 128])
                            mirror_src[dst] = sb

                def softmax_row(ct):
                    for dt in range(ct):
                        nc.tensor.transpose(
                            ep[ct][:, dt * 128:(dt + 1) * 128],
                            mirror_src[(ct, dt)], ident,
                        )
                    mn = psmall.tile([128, 1], f32, tag="mn")
                    nc.vector.tensor_reduce(
                        out=mn, in_=ep[ct], axis=mybir.AxisListType.X,
                        op=mybir.AluOpType.min,
                    )
                    a_ = patt.tile([128, C], f16, tag=f"att{ct}")
                    ss = psmall.tile([128, 1], f32, tag="ss")
                    nc.scalar.activation(
                        out=a_, in_=ep[ct],
                        func=mybir.ActivationFunctionType.Exp,
                        bias=mn, scale=-1.0, accum_out=ss,
                    )
                    rg = psmall.tile([128, 1], f32, tag="rg")
                    nc.vector.reciprocal(out=rg, in_=ss)
                    nc.vector.tensor_mul(out=rg, in0=rg, in1=gam)
                    nc.vector.tensor_scalar_mul(a_, a_, rg)
                    att[ct] = a_

                ep[0] = pep.tile([128, C], f32, tag="ep", name="ep0")
                ep[1] = pep.tile([128, C], f32, tag="ep", name="ep1")
                v16q = {}
                for q in range(4):
                    # conversions first so every engine starts its quarter
                    # conversion as soon as the load lands
                    for ct in range(CT):
                        v16 = pv16.tile([128, QW], f16, tag=f"v16_{ct % 2}",
                                        name=f"v16_{ct % 2}")
                        _copy(nc, V16_E[q * 4 + ct], v16, v[ct][q])
                        v16q[ct] = v16
                    yield
                    for ct in range(CT):
                        tp = ptp.tile([128, 8, 128], f16, tag="tp")
                        for ks in range(8):
                            nc.tensor.transpose(
                                tp[:, ks, :],
                                v16q[ct][:, ks * 128:(ks + 1) * 128],
                                ident16,
                            )
                        _copy(nc, VT_E[q * 4 + ct],
                              vTq[q][:, :, ct * 128:(ct + 1) * 128], tp)
                    if q == 0 and bi == 0:
                        load_gamma()
                    if q == 0 and bi + 1 < len(batches):
                        state["v"] = load_quarters(
                            batches[bi + 1],
                            [[None] * 4 for _ in range(CT)],
                            range(4), spread=False)
                    yield
                    e_row(0, range(8 * q, 8 * q + 8))
                    yield
                    e_row(1, range(8 * q, 8 * q + 8))
                    yield
                    if inline_v8 and q % 2 == 1:
                        v8_emit(out, q - 1)
                        v8_emit(out, q)
                        yield


                stash_mirrors(0)
                softmax_row(0)
                stash_mirrors(1)
                softmax_row(1)
                yield
                ep[2] = pep.tile([128, C], f32, tag="ep", name="ep2")
                e_row(2, range(KCH))
                stash_mirrors(2)
                softmax_row(2)
                yield
                ep[3] = pep.tile([128, C], f32, tag="ep", name="ep3")
                e_row(3, range(KCH))
                softmax_row(3)
                yield

            def a_emit(te):
                """A phase: att -> fp8 attT pair tiles; called after the
                previous batch's O chunks so they fill the softmax wait."""
                att = te["att"]
                aT8 = [pat8.tile([128, 2, CT, 128], f8, tag=f"aT8_{p}",
                                 name=f"aT8_{p}") for p in range(2)]
                te["aT8"] = aT8
                for dt in range(CT):
                    atp = ptp.tile([128, 8, 128], f16, tag="tp")
                    for ct in range(CT):
                        nc.tensor.transpose(
                            atp[:, ct, :],
                            att[ct][:, dt * 128:(dt + 1) * 128],
                            ident16,
                        )
                    _copy(nc, AT8_E[dt],
                          aT8[dt // 2][:, dt % 2, :, :], atp[:, :CT, :])

            def o_chunks(b, te):
                """Yield O-phase 1024-wide chunks for batch b."""
                v, v8, aT8 = te["v"], te["v8"], te["aT8"]

                for ct in (3, 2, 0, 1):
                    for nn in range(NN):
                        eng, resid = EPI_SEQ[(ct + nn) % 2]
                        op = pop.tile([128, 1024], f32, tag="op")
                        for half in range(2):
                            osl = slice(half * 512, (half + 1) * 512)
                            for p in range(2):
                                nc.tensor.matmul(
                                    op[:, osl],
                                    lhsT=aT8[p][:, :, ct, :],
                                    rhs=v8[p][:, :, nn * 1024 + half * 512:
                                              nn * 1024 + half * 512 + 512],
                                    start=(p == 0),
                                    stop=(p == 1 and not resid),
                                    perf_mode=DR,
                                    skip_group_check=True,
                                )
                            if resid:
                                nc.tensor.matmul(
                                    op[:, osl],
                                    lhsT=ident,
                                    rhs=v[ct][nn][:, osl],
                                    start=False,
                                    stop=True,
                                    skip_group_check=True,
                                )
                        st = pstage.tile([128, 1024], f32, tag="st")
                        if resid:
                            nc.scalar.copy(out=st, in_=op)
                        else:
                            nc.vector.tensor_add(out=st, in0=op, in1=v[ct][nn])
                        nsl = slice(nn * 1024, (nn + 1) * 1024)
                        getattr(nc, STORE_Q[ct][nn]).dma_start(
                            out=y_d[b, ct * 128:(ct + 1) * 128, nsl], in_=st,
                        )
                        yield

            def pump(g):
                """Advance a generator; True while it has more to emit."""
                try:
                    next(g)
                    return True
                except StopIteration:
                    return False

            prev_o = None
            for bi, b in enumerate(batches):
                te = {}
                gen = te_quanta(bi, b, te, inline_v8=(bi == 0))
                o_more = prev_o is not None
                v8_todo = bi > 0
                for _ in gen:
                    if o_more:
                        o_more = pump(prev_o)
                    if not o_more and v8_todo:
                        # previous O fully emitted: v8 copies can go now
                        # without head-blocking any engine queue
                        for qq in range(4):
                            v8_emit(te, qq)
                        v8_todo = False
                while o_more:
                    o_more = pump(prev_o)
                if v8_todo:
                    for qq in range(4):
                        v8_emit(te, qq)
                a_emit(te)
                prev_o = o_chunks(b, te)
            for _ in prev_o:
                pass

    nc.compile()
    return nc


def kernel(x: np.ndarray, gamma: np.ndarray) -> np.ndarray:
    x = np.ascontiguousarray(np.asarray(x, dtype=np.float32))
    gamma = np.ascontiguousarray(np.asarray(gamma, dtype=np.float32))
    B, Cc, H, W = x.shape
    xv = x.reshape(B, Cc, H * W)

    if "nc" not in _CACHE:
        _CACHE["nc"] = _build_nc()
    nc = _CACHE["nc"]

    in_maps = [
        {"x": xv[i * B_PER_CORE:(i + 1) * B_PER_CORE], "gamma": gamma}
        for i in range(N_CORES)
    ]
    res = run_bass_kernel_spmd(nc, in_maps, list(range(N_CORES)))
    y = np.concatenate([res.results[i]["y"] for i in range(N_CORES)], axis=0)
    return y.reshape(B, Cc, H, W).astype(np.float32)


# revision 65
# speedup vs baseline: 1.1171x; 1.0142x over previous
"""CAM (channel attention module) kernel for Trainium2, 8-core SPMD.

Problem: x (16, 512, 64, 64) f32, gamma (1,) f32.
  v = x.reshape(B, C, N);  E = v @ v.T  (B x 512 x 512)
  att = softmax(rowmax(E) - E)  ==  exp(rowmin(E) - E) / rowsum(...)
  out = gamma * (att @ v) + x
Sharding: data-parallel over batch, 2 batches per core, no collectives.

Cost-model facts this schedule is built around (CoreSim legacy model):
  - Each DMA queue is per-engine and serial WITH that engine's compute
    track; cost = bytes-per-partition * 0.386ns. SP has no compute: it
    carries most stores and ALL of the next batch's prefetched loads
    (so prefetch never blocks the ACT/Pool compute tracks).
  - Hardware BIR rules honored here that CoreSim does not check: GPSIMD
    may never read/write PSUM, and fp32r matmul operands must come from a
    rounding instruction (so the x residual rides the DVE add instead).
  - PE matmul cost = out_free * cycles/row (fp16 1.0, fp8e4 DoubleRow 0.5
    with a 256-deep contraction, fp32r 1.0 at free>=256, transpose fp16
    1.0). PE total is ~73us for 2 batches - the critical path.
  - Engine elementwise cost ~ elements * cycle (DVE 1.04, ACT 0.83, Pool
    1.39 effective). fp16->fp16 copies get 2x on DVE.

Per-core per-batch pipeline (quarter-granular so the conversion feed
always runs ahead of the in-order PE):
  T: DMA v quarters (f32 [128,1024], q-major, round-robin queues;
     next batch prefetched on sync during this batch's T phase) ->
     f32->fp16 quarter copies (DVE/ACT/Pool round-robin) -> 8 PE
     transposes -> one copy into that quarter's vT tile.
  E: energy = vT.T @ vT, fp16 operands, f32 PSUM accumulation, upper
     triangle only; the 6 lower blocks are mirrored from earlier rows by
     PE transpose back into the PSUM row. Rows 0/1 stream per quarter
     right behind the transposes; rows 2/3 + mirrors run after quarter 3,
     reusing rows 0/1's PSUM banks.
  S: fused softmax of (rowmin(E) - E): DVE rowmin, ACT exp with rowsum,
     DVE reciprocal; gamma is folded into the per-row scale, so gamma==0
     gives att==0 and the output is exactly x.
  A: per d-tile: 4 PE transposes att -> PSUM (fp16) + one copy that
     downcasts into fp8e4 pair tiles (two d-tiles packed for DoubleRow).
     Emitted after the previous batch's leftover O chunks so those fill
     the softmax wait on the in-order PE.
  O: per (ct, 1024-wide column pair): two 512-wide PSUM groups, each two
     fp8e4 DoubleRow matmuls over d=512. The epilogue is a DVE fused
     tensor_add (op + exact-f32 x) -> SBUF staging (GPSIMD cannot touch
     PSUM on hardware, and ACT cannot tensor-add, so DVE owns it; ACT
     instead owns half the vT copies + the exp). Stores spread over
     sync/scalar/gpsimd, with the final two chunks split into 512-wide
     stores on two queues to shorten the drain tail.
  O(b) is interleaved with TE(b+1) quanta at emission (TE quantum first,
     so the next batch's chain-critical conversions get queue priority
     over the epilogue, which has slack).
"""
import sys

import numpy as np

if "/opt/trn_rl_repo" not in sys.path:
    sys.path.insert(0, "/opt/trn_rl_repo")

import concourse.bass as bass
import concourse.tile as tile
from concourse import bacc, mybir
from concourse.bass_utils import run_bass_kernel_spmd
from concourse.masks import make_identity

N_CORES = 8
B_FULL = 16
B_PER_CORE = B_FULL // N_CORES  # 2
C = 512            # channels
HW = 4096          # H*W
CT = C // 128      # 4 channel tiles
KCH = HW // 128    # 32 contraction chunks for energy
QW = HW // 4       # quarter of H*W
NN = 4             # 1024-wide output column pairs per row tile

f32 = mybir.dt.float32
f32r = mybir.dt.float32r
f16 = mybir.dt.float16
f8 = mybir.dt.float8e4
DR = mybir.MatmulPerfMode.DoubleRow

_CACHE = {}

# DMA queue per (ct, q) for this batch's loads; next batch: quarters 0/1
# prefetched on sync, quarters 2/3 spread late (after this batch's
# conversions clear the ACT/Pool queues)
LOAD_Q = ("sync", "scalar", "gpsimd", "sync")
# f32->fp16 quarter conversion engine per (q*4+ct)
V16_E = ("vector", "gpsimd", "scalar", "scalar",
         "gpsimd", "scalar", "vector", "gpsimd",
         "scalar", "gpsimd", "scalar", "scalar",
         "gpsimd", "scalar", "vector", "gpsimd")
# vT PSUM->SBUF copy engine per (q*4+ct)
VT_E = ("vector", "scalar", "vector", "scalar",
        "scalar", "vector", "scalar", "vector",
        "vector", "scalar", "vector", "scalar",
        "scalar", "vector", "scalar", "vector")
# f32->fp8 quarter conversion engine per (q*4+ct) (not chain-critical)
V8_E = ("gpsimd", "scalar", "gpsimd", "gpsimd",
        "scalar", "gpsimd", "gpsimd", "scalar",
        "gpsimd", "gpsimd", "scalar", "gpsimd",
        "gpsimd", "scalar", "gpsimd", "gpsimd")
# epilogue per (ct + nn) % 2: DVE does fused tensor_add(op + x); ACT
# chunks accumulate the exact-f32 x residual via an fp32r identity matmul
# on PE then do a plain PSUM->SBUF copy (GPSIMD cannot touch PSUM, and
# ACT cannot tensor-add, so att is pre-scaled by gamma/rowsum on DVE)
EPI_SEQ = (("vector", False), ("vector", False))
# store queue per (ct, nn)
STORE_Q = {3: ("scalar", "sync", "gpsimd", "sync"),
           0: ("sync", "gpsimd", "scalar", "sync"),
           1: ("sync", "sync", "scalar", "gpsimd"),
           2: ("sync", "scalar", "gpsimd", "sync")}
MIR_E = "vector"  # mirror stashes read PSUM: GPSIMD cannot
AT8_E = ("scalar", "vector", "scalar", "vector")  # attT fp8 pack per dt


def _copy(nc, eng, out, in_):
    if eng == "scalar":
        nc.scalar.copy(out=out, in_=in_)
    else:
        getattr(nc, eng).tensor_copy(out=out, in_=in_)


def _build_nc(reps: int = 1):
    nc = bacc.Bacc(None, target_bir_lowering=False)
    x_d = nc.dram_tensor("x", [B_PER_CORE, C, HW], f32, kind="ExternalInput")
    g_d = nc.dram_tensor("gamma", [1], f32, kind="ExternalInput")
    y_d = nc.dram_tensor("y", [B_PER_CORE, C, HW], f32, kind="ExternalOutput")

    with tile.TileContext(nc) as tc:
        with (
            tc.tile_pool(name="pv", bufs=2) as pv,          # v quarters f32
            tc.tile_pool(name="pvt", bufs=1) as pvt,        # vT fp16
            tc.tile_pool(name="pv16", bufs=1) as pv16,      # fp16 v quarters
            tc.tile_pool(name="patt", bufs=1) as patt,      # att fp16
            tc.tile_pool(name="pat8", bufs=1) as pat8,      # attT fp8 pairs
            tc.tile_pool(name="pv8", bufs=1) as pv8,        # v fp8 pairs
            tc.tile_pool(name="pstage", bufs=4) as pstage,  # out staging
            tc.tile_pool(name="psmall", bufs=8) as psmall,  # per-ct scalars
            tc.tile_pool(name="pmir", bufs=1) as pmir,      # mirror blocks
            tc.tile_pool(name="psing", bufs=1) as psing,    # ident, gamma
            tc.tile_pool(name="ptp", bufs=2, space="PSUM") as ptp,
            tc.tile_pool(name="pep", bufs=2, space="PSUM") as pep,
            tc.tile_pool(name="pop", bufs=2, space="PSUM") as pop,
        ):
            ident = psing.tile([128, 128], f32)
            make_identity(nc, ident)
            ident16 = psing.tile([128, 128], f16)
            nc.vector.tensor_copy(out=ident16, in_=ident)
            gam = psing.tile([128, 1], f32)

            def load_gamma():
                g_ap = g_d[:]
                nc.gpsimd.dma_start(
                    out=gam,
                    in_=bass.AP(tensor=g_ap.tensor, offset=g_ap.offset,
                                ap=[[0, 128], [1, 1]]),
                )

            def load_quarters(b, tiles, qs, spread):
                for q in qs:
                    for ct in range(CT):
                        t_ = pv.tile([128, QW], f32, tag=f"v{ct}q{q}",
                                     name=f"v{ct}q{q}")
                        q_ = LOAD_Q[(q + ct) % len(LOAD_Q)] if spread \
                            else "sync"
                        getattr(nc, q_).dma_start(
                            out=t_,
                            in_=x_d[b, ct * 128:(ct + 1) * 128,
                                    q * QW:(q + 1) * QW],
                        )
                        tiles[ct][q] = t_
                return tiles

            def load_group(b, spread=True):
                return load_quarters(
                    b, [[None] * 4 for _ in range(CT)], range(4), spread)

            batches = [bb for _ in range(reps) for bb in range(B_PER_CORE)]
            state = {"v": load_group(batches[0])}

            def v8_emit(te, qq):
                """fp8 pair copies of v quarter qq for the O phase. The
                tiles are allocated HERE (first call), after the previous
                batch's O chunks are fully emitted, so the pool's
                write-after-read hazard tracking sees every reader of the
                previous tiles before the new writers are scheduled."""
                v = te["v"]
                if te["v8"] is None:
                    te["v8"] = [pv8.tile([128, 2, HW], f8, tag=f"v8_{p_}",
                                         name=f"v8_{p_}") for p_ in range(2)]
                v8 = te["v8"]
                for ct in range(CT):
                    _copy(nc, V8_E[qq * 4 + ct],
                          v8[ct // 2][:, ct % 2, qq * QW:(qq + 1) * QW],
                          v[ct][qq])

            def te_quanta(bi, b, out, inline_v8):
                """Emit T+E+S for batch b, yielding at quantum boundaries
                for interleaving with the previous O phase."""
                v = state["v"]
                out["v"] = v
                vTq = [pvt.tile([128, 8, C], f16, tag=f"vTq{q}",
                                name=f"vTq{q}") for q in range(4)]

                def vT(k):
                    return vTq[k // 8][:, k % 8, :]

                out["v8"] = None
                ep = {}
                att = [None] * CT
                out["att"] = att
                mirror_src = {}

                def e_row(ct, ks):
                    off = ct * 128
                    for k in ks:
                        nc.tensor.matmul(
                            ep[ct][:, off:],
                            lhsT=vT(k)[:, off:off + 128],
                            rhs=vT(k)[:, off:],
                            start=(k == 0),
                            stop=(k == KCH - 1),
                        )

                def stash_mirrors(ct):
                    for (dst, src) in (((1, 0), (0, 1)), ((2, 0), (0, 2)),
                                       ((2, 1), (1, 2)), ((3, 0), (0, 3)),
                                       ((3, 1), (1, 3)), ((3, 2), (2, 3))):
                        if src[0] == ct:
                            sb = pmir.tile([128, 128], f32,
                                           tag=f"mir{dst[0]}{dst[1]}")
                            _copy(nc, MIR_E, sb,
                                  ep[ct][:, src[1] * 128:(src[1] + 1) * 128])
                            mirror_src[dst] = sb

                def softmax_row(ct):
                    for dt in range(ct):
                        nc.tensor.transpose(
                            ep[ct][:, dt * 128:(dt + 1) * 128],
                            mirror_src[(ct, dt)], ident,
                        )
                    mn = psmall.tile([128, 1], f32, tag="mn")
                    nc.vector.tensor_reduce(
                        out=mn, in_=ep[ct], axis=mybir.AxisListType.X,
                        op=mybir.AluOpType.min,
                    )
                    a_ = patt.tile([128, C], f16, tag=f"att{ct}")
                    ss = psmall.tile([128, 1], f32, tag="ss")
                    nc.scalar.activation(
                        out=a_, in_=ep[ct],
                        func=mybir.ActivationFunctionType.Exp,
                        bias=mn, scale=-1.0, accum_out=ss,
                    )
                    rg = psmall.tile([128, 1], f32, tag="rg")
                    nc.vector.reciprocal(out=rg, in_=ss)
                    nc.vector.tensor_mul(out=rg, in0=rg, in1=gam)
                    nc.vector.tensor_scalar_mul(a_, a_, rg)
                    att[ct] = a_

                ep[0] = pep.tile([128, C], f32, tag="ep", name="ep0")
                ep[1] = pep.tile([128, C], f32, tag="ep", name="ep1")
                v16q = {}
                for q in range(4):
                    # conversions first so every engine starts its quarter
                    # conversion as soon as the load lands
                    for ct in range(CT):
                        v16 = pv16.tile([128, QW], f16, tag=f"v16_{ct % 2}",
                                        name=f"v16_{ct % 2}")
                        _copy(nc, V16_E[q * 4 + ct], v16, v[ct][q])
                        v16q[ct] = v16
                    yield
                    for ct in range(CT):
                        tp = ptp.tile([128, 8, 128], f16, tag="tp")
                        for ks in range(8):
                            nc.tensor.transpose(
                                tp[:, ks, :],
                                v16q[ct][:, ks * 128:(ks + 1) * 128],
                                ident16,
                            )
                        _copy(nc, VT_E[q * 4 + ct],
                              vTq[q][:, :, ct * 128:(ct + 1) * 128], tp)
                    if q == 0 and bi == 0:
                        load_gamma()
                    if q == 0 and bi + 1 < len(batches):
                        state["v"] = load_quarters(
                            batches[bi + 1],
                            [[None] * 4 for _ in range(CT)],
                            range(4), spread=False)
                    yield
                    e_row(0, range(8 * q, 8 * q + 8))
                    yield
                    e_row(1, range(8 * q, 8 * q + 8))
                    yield
                    if inline_v8 and q % 2 == 1:
                        v8_emit(out, q - 1)
                        v8_emit(out, q)
                        yield


                stash_mirrors(0)
                softmax_row(0)
                stash_mirrors(1)
                softmax_row(1)
                yield
                ep[2] = pep.tile([128, C], f32, tag="ep", name="ep2")
                e_row(2, range(KCH))
                stash_mirrors(2)
                softmax_row(2)
                yield
                ep[3] = pep.tile([128, C], f32, tag="ep", name="ep3")
                e_row(3, range(KCH))
                softmax_row(3)
                yield

            def a_emit(te):
                """A phase: att -> fp8 attT pair tiles; called after the
                previous batch's O chunks so they fill the softmax wait."""
                att = te["att"]
                aT8 = [pat8.tile([128, 2, CT, 128], f8, tag=f"aT8_{p}",
                                 name=f"aT8_{p}") for p in range(2)]
                te["aT8"] = aT8
                for dt in range(CT):
                    atp = ptp.tile([128, 8, 128], f16, tag="tp")
                    for ct in range(CT):
                        nc.tensor.transpose(
                            atp[:, ct, :],
                            att[ct][:, dt * 128:(dt + 1) * 128],
                            ident16,
                        )
                    _copy(nc, AT8_E[dt],
                          aT8[dt // 2][:, dt % 2, :, :], atp[:, :CT, :])

            def o_chunks(b, te, last=False):
                """Yield O-phase 1024-wide chunks for batch b."""
                v, v8, aT8 = te["v"], te["v8"], te["aT8"]

                for ct in (3, 2, 0, 1):
                    for nn in range(NN):
                        eng, resid = EPI_SEQ[(ct + nn) % 2]
                        op = pop.tile([128, 1024], f32, tag="op")
                        for half in range(2):
                            osl = slice(half * 512, (half + 1) * 512)
                            for p in range(2):
                                nc.tensor.matmul(
                                    op[:, osl],
                                    lhsT=aT8[p][:, :, ct, :],
                                    rhs=v8[p][:, :, nn * 1024 + half * 512:
                                              nn * 1024 + half * 512 + 512],
                                    start=(p == 0),
                                    stop=(p == 1 and not resid),
                                    perf_mode=DR,
                                    skip_group_check=True,
                                )
                            if resid:
                                nc.tensor.matmul(
                                    op[:, osl],
                                    lhsT=ident,
                                    rhs=v[ct][nn][:, osl],
                                    start=False,
                                    stop=True,
                                    skip_group_check=True,
                                )
                        st = pstage.tile([128, 1024], f32, tag="st")
                        if resid:
                            nc.scalar.copy(out=st, in_=op)
                        else:
                            nc.vector.tensor_add(out=st, in0=op, in1=v[ct][nn])
                        nsl = slice(nn * 1024, (nn + 1) * 1024)
                        if last and ct == 1 and nn >= 2:
                            # split the final stores across two queues so
                            # the drain tail shrinks by a store slot
                            h0 = slice(nn * 1024, nn * 1024 + 512)
                            h1 = slice(nn * 1024 + 512, (nn + 1) * 1024)
                            qa, qb = (("sync", "gpsimd") if nn == 2
                                      else ("scalar", "sync"))
                            getattr(nc, qa).dma_start(
                                out=y_d[b, ct * 128:(ct + 1) * 128, h0],
                                in_=st[:, 0:512])
                            getattr(nc, qb).dma_start(
                                out=y_d[b, ct * 128:(ct + 1) * 128, h1],
                                in_=st[:, 512:1024])
                        else:
                            getattr(nc, STORE_Q[ct][nn]).dma_start(
                                out=y_d[b, ct * 128:(ct + 1) * 128, nsl],
                                in_=st,
                            )
                        yield

            def pump(g):
                """Advance a generator; True while it has more to emit."""
                try:
                    next(g)
                    return True
                except StopIteration:
                    return False

            prev_o = None
            for bi, b in enumerate(batches):
                te = {}
                gen = te_quanta(bi, b, te, inline_v8=(bi == 0))
                o_more = prev_o is not None
                v8_todo = bi > 0
                yi = 0
                for _ in gen:
                    # let the next batch's first conversions claim their
                    # engine-queue slots before the previous O drain starts
                    yi += 1
                    if o_more and yi > 3:
                        o_more = pump(prev_o)
                    if not o_more and v8_todo:
                        # previous O fully emitted: v8 copies can go now
                        # without head-blocking any engine queue
                        for qq in range(4):
                            v8_emit(te, qq)
                        v8_todo = False
                while o_more:
                    o_more = pump(prev_o)
                if v8_todo:
                    for qq in range(4):
                        v8_emit(te, qq)
                a_emit(te)
                prev_o = o_chunks(b, te, last=(bi == len(batches) - 1))
            for _ in prev_o:
                pass

    nc.compile()
    return nc


def kernel(x: np.ndarray, gamma: np.ndarray) -> np.ndarray:
    x = np.ascontiguousarray(np.asarray(x, dtype=np.float32))
    gamma = np.ascontiguousarray(np.asarray(gamma, dtype=np.float32))
    B, Cc, H, W = x.shape
    xv = x.reshape(B, Cc, H * W)

    if "nc" not in _CACHE:
        _CACHE["nc"] = _build_nc()
    nc = _CACHE["nc"]

    in_maps = [
        {"x": xv[i * B_PER_CORE:(i + 1) * B_PER_CORE], "gamma": gamma}
        for i in range(N_CORES)
    ]
    res = run_bass_kernel_spmd(nc, in_maps, list(range(N_CORES)))
    y = np.concatenate([res.results[i]["y"] for i in range(N_CORES)], axis=0)
    return y.reshape(B, Cc, H, W).astype(np.float32)
